# revision 14
# baseline (speedup 1.0000x reference)
"""Trainium2 Bass kernel for DifferentiableSupergraphDynamics.

Computation:
    edge_w = where(learn_mask, tanh(theta), sign*conf) * delay      [E]
    msgs   = x[:, src] * edge_w                                     [B, E]
    agg    = scatter_add(msgs -> dst)                               [B, N]
    rate   = base_rate * exp(rate_log_scale)                        [N]
    drive  = tanh(agg + bias)
    x_next = clip(x + DT * rate * (drive*cap - x), 0, cap)

Design (v2):
  - Destination nodes are dealt round-robin by total-degree rank across the
    8 cores; every edge lives on its destination's core (no collective).
  - Single-structure gather: the x table is packed as [N/4, 4*B] f32 (4 node
    rows per 256B line).  Each edge's SWDGE descriptor fetches the full 256B
    line holding its source row, so the int16 gather index (= src//4 <
    32768) reaches the whole table and no src-range structure split / merge
    scatter is needed.  Row selection happens in the weighted reduce: a
    host-built [4]-wide weight mask per slot (w at position src%4, else 0).
  - CSR: per-core nodes sorted by degree, groups of 128 partitions padded to
    the group max degree D (D shared across cores).  The slot space is cut
    into uniform STRIP_COLS-column slabs (2048-slot gather calls, SWDGE
    queues round-robin 1,2,3,0 -- equal sizes keep queue 0's
    pipeline-blocking ucode hidden under queues 1-3's generation).  Per
    slab, one DVE mask-multiply and one fused (sub-row x degree-window)
    reduce run under later slabs' gathers; group windows straddling a slab
    boundary reduce each piece directly into a per-piece-index shadow
    aggregate (no read-modify-write), merged once at the end.
  - Edge weights and the epilogue's affine terms are precomputed on host:
    out = clip(A + C*tanh(agg + bias), 0, cap) with A=(1-DT*rate)*x and
    C = DT*rate*cap.  The epilogue runs in two chunks (groups finalized by
    mid-kernel, rest at the end).
"""

import numpy as np

import concourse.bass as bass
import concourse.bacc as bacc
import concourse.mybir as mybir
import concourse.tile as tile
from concourse.bass_utils import run_bass_kernel_spmd

P = 128
NCORES = 8
DT = 0.1
EPS = 1e-5
STRIP_COLS = 16            # slab width in columns (4096-slot gather calls)
MAX_CALL = 8192            # SWDGE ring capacity per call

F32 = mybir.dt.float32
I16 = mybir.dt.int16


def _wrap_idx(flat):
    """SWDGE wrapped int16 index layout for one call: index j at
    [j%16, j//16], replicated to 128 partitions."""
    n = len(flat)
    assert n % 16 == 0
    cols = flat.reshape(n // 16, 16).T
    return np.concatenate([cols] * 8, axis=0)


# ---------------------------------------------------------------------------
# Host-side data preparation
# ---------------------------------------------------------------------------

def _prep(x, theta, bias, ratelog, baserate, cap, sign, conf, delay, src, dst,
          mask, n_cores):
    B, N = x.shape
    E = src.shape[0]

    src = np.asarray(src).astype(np.int64)
    dst = np.asarray(dst).astype(np.int64)
    x = np.asarray(x, dtype=np.float32)

    # host-computed edge weight
    w = np.where(np.asarray(mask).astype(bool),
                 np.tanh(np.asarray(theta, dtype=np.float32)),
                 np.asarray(sign, dtype=np.float32) *
                 np.asarray(conf, dtype=np.float32)) \
        * np.asarray(delay, dtype=np.float32)

    deg = np.bincount(dst, minlength=N)
    order = np.argsort(-deg, kind="stable")
    rank_of = np.empty(N, dtype=np.int64)
    rank_of[order] = np.arange(N)
    core_of = rank_of % n_cores
    pos_of = rank_of // n_cores
    npc = (N + n_cores - 1) // n_cores
    G = (npc + P - 1) // P
    nper = G * P

    # shared-over-cores group degree D[g] = max over cores of group max
    degs = np.zeros((n_cores, nper), dtype=np.int64)
    for c in range(n_cores):
        dc = deg[order[c::n_cores]]
        degs[c, :len(dc)] = dc
    D = degs.reshape(n_cores, G, P).max(axis=(0, 2))
    S = np.zeros(G + 1, dtype=np.int64)
    S[1:] = np.cumsum(D)
    F = int(S[-1])

    # uniform slabs of STRIP_COLS columns (cut anywhere; group windows that
    # straddle a slab boundary are reduced partially and accumulated)
    slabs = []
    piece_no = {}
    A = 0
    while A < F:
        wcols = STRIP_COLS if A < F - 128 else STRIP_COLS // 2
        Bc = min(A + wcols, F)
        runs = []          # (ga, gb, d): full windows, equal-d batches
        partials = []      # (g, a, b): partial window cols [a, b)
        fulls = []
        for g in range(G):
            if D[g] == 0 or S[g] >= Bc or S[g + 1] <= A:
                continue
            a, b = max(int(S[g]), A), min(int(S[g + 1]), Bc)
            if a == S[g] and b == S[g + 1]:
                fulls.append(g)
            else:
                k = piece_no.get(g, 0)
                piece_no[g] = k + 1
                partials.append((g, a, b, k))
        i = 0
        while i < len(fulls):
            j = i + 1
            while (j < len(fulls) and fulls[j] == fulls[j - 1] + 1
                   and D[fulls[j]] == D[fulls[i]]):
                j += 1
            runs.append((fulls[i], fulls[j - 1] + 1, int(D[fulls[i]])))
            i = j
        slabs.append(dict(A=A, B=Bc, runs=runs, partials=partials))
        A = Bc

    # edge -> slot
    ec = core_of[dst]
    ep = pos_of[dst]
    eord = np.argsort(ec * nper + ep, kind="stable")
    key = (ec * nper + ep)[eord]
    key_change = np.ones(E, dtype=bool)
    key_change[1:] = key[1:] != key[:-1]
    run_starts = np.flatnonzero(key_change)
    occ = np.arange(E) - run_starts[np.cumsum(key_change) - 1]
    g = ep[eord] // P
    pp = ep[eord] % P
    col = S[g] + occ
    slot_i = pp + P * col

    n4 = (N + 3) // 4
    srcg = (src[eord] // 4).astype(np.int16)
    subr = (src[eord] % 4).astype(np.int64)
    assert n4 <= 32768

    idxA = np.zeros((n_cores, F * P), np.int16)
    w4A = np.zeros((n_cores, P, F, 4), np.float32)
    ecs = ec[eord]
    idxA[ecs, slot_i] = srcg
    w4A[ecs, pp, col, subr] = w[eord]

    # wrapped gather indices (whole array; slab slices are column slices)
    gidx = np.zeros((n_cores, 128, F * 8), np.int16)
    for c in range(n_cores):
        gidx[c] = _wrap_idx(idxA[c])

    # node params in canonical [P, G] placement
    rate = np.asarray(baserate, dtype=np.float32) * \
        np.exp(np.asarray(ratelog, dtype=np.float32))
    Cv = DT * rate * np.asarray(cap, dtype=np.float32)
    Av = (1.0 - DT * rate)[None, :] * x            # [B, N]

    biasA = np.zeros((n_cores, P, G), np.float32)
    CA = np.zeros((n_cores, P, G), np.float32)
    capA = np.ones((n_cores, P, G), np.float32)
    AA = np.zeros((n_cores, P, G, B), np.float32)
    node_ids = np.full((n_cores, P, G), -1, np.int64)
    biasv = np.asarray(bias, dtype=np.float32)
    capv = np.asarray(cap, dtype=np.float32)
    for c in range(n_cores):
        nd = order[c::n_cores]                     # nodes at pos 0..len-1
        j = np.arange(len(nd))
        pidx = (j % P, j // P)
        node_ids[c][pidx] = nd
        biasA[c][pidx] = biasv[nd]
        CA[c][pidx] = Cv[nd]
        capA[c][pidx] = capv[nd]
        AA[c][pidx[0], pidx[1], :] = Av[:, nd].T

    xq = np.zeros((n4, 4 * B), np.float32)
    xq.reshape(-1, B)[:N] = x.T

    ins = []
    for c in range(n_cores):
        ins.append({
            "xq": xq,
            "gidx": gidx[c],
            "w4": w4A[c].reshape(P, F * 4),
            "bias": biasA[c],
            "cmul": CA[c],
            "cap": capA[c],
            "apre": AA[c].reshape(P, G * B),
        })
    nshadow = max(piece_no.values(), default=1)
    # epilogue chunks: after slab `si`, groups [g_lo, g_hi) are final
    sends = np.searchsorted([sl["B"] for sl in slabs],
                            np.maximum(S[1:] - 1, S[:-1]), side="right")
    sends[D == 0] = len(slabs) - 1
    nch = 2
    marks = sorted(set(round((i + 1) * len(slabs) / nch) - 1
                       for i in range(nch)))
    echunks = []
    g_done = 0
    for mk in marks:
        hi = int(np.searchsorted(sends, mk, side="right"))
        if mk == len(slabs) - 1:
            hi = G
        if hi > g_done:
            echunks.append((int(mk), g_done, hi))
            g_done = hi
    plan = dict(B=B, G=G, F=F, D=D, S=S, slabs=slabs, n4=n4,
                nshadow=nshadow, echunks=echunks, node_ids=node_ids)
    return ins, plan


def _assemble(results, plan):
    B, G = plan["B"], plan["G"]
    N = 0
    for nid in plan["node_ids"]:
        N = max(N, nid.max() + 1)
    out = np.empty((B, N), dtype=np.float32)
    for ci, res in enumerate(results):
        o = res["out"].reshape(P, G, B)
        nid = plan["node_ids"][ci]
        ok = nid >= 0
        out[:, nid[ok]] = o[ok].T
    return out


# ---------------------------------------------------------------------------
# Device kernel
# ---------------------------------------------------------------------------

def _equal_d_runs(D, g0, g1):
    runs = []
    a = g0
    while a < g1:
        b = a + 1
        while b < g1 and D[b] == D[a]:
            b += 1
        runs.append((a, b, int(D[a])))
        a = b
    return runs


def build(B, G, F, D, S, slabs, n4, nshadow=2, echunks=None):
    nc = bacc.Bacc("TRN2", target_bir_lowering=False, debug=False,
                   enable_asserts=False, num_swdge_queues=4)

    xqD = nc.dram_tensor("xq", [n4, 4 * B], F32, kind="ExternalInput")
    giD = nc.dram_tensor("gidx", [128, F * 8], I16, kind="ExternalInput")
    w4D = nc.dram_tensor("w4", [P, F * 4], F32, kind="ExternalInput")
    biD = nc.dram_tensor("bias", [P, G], F32, kind="ExternalInput")
    cmD = nc.dram_tensor("cmul", [P, G], F32, kind="ExternalInput")
    cpD = nc.dram_tensor("cap", [P, G], F32, kind="ExternalInput")
    apD = nc.dram_tensor("apre", [P, G * B], F32, kind="ExternalInput")
    outD = nc.dram_tensor("out", [P, G * B], F32, kind="ExternalOutput")

    Tanh = mybir.ActivationFunctionType.Tanh
    qorder = [1, 2, 3, 0]

    with tile.TileContext(nc) as tc:
        with (
            tc.tile_pool(name="persist", bufs=1) as ppool,
            tc.tile_pool(name="strip", bufs=12) as sp,
        ):
            agg = ppool.tile([P, G * B], F32, tag="agg")
            nc.vector.memset(agg[:], 0.0)
            shadows = []
            for k in range(nshadow):
                sh = ppool.tile([P, G * B], F32, tag=f"shadow{k}")
                nc.vector.memset(sh[:], 0.0)
                shadows.append(sh)

            # epilogue params up front so the tail never waits on DMA
            bi = ppool.tile([P, G], F32, tag="bi")
            cm = ppool.tile([P, G], F32, tag="cm")
            cp = ppool.tile([P, G], F32, tag="cp")
            ap_ = ppool.tile([P, G * B], F32, tag="ap")

            for si, sl in enumerate(slabs):
                A, Bc = sl["A"], sl["B"]
                sc = Bc - A
                nidx = sc * P
                gt = sp.tile([128, sc * 8], I16, tag="gidx")
                nc.sync.dma_start(out=gt[:], in_=giD[:, A * 8:Bc * 8])
                wt = sp.tile([P, sc * 4], F32, tag="w4")
                nc.sync.dma_start(out=wt[:], in_=w4D[:, A * 4:Bc * 4])
                if si == 40:
                    nc.sync.dma_start(out=bi[:], in_=biD[:, :])
                    nc.sync.dma_start(out=cm[:], in_=cmD[:, :])
                    nc.sync.dma_start(out=cp[:], in_=cpD[:, :])
                    nc.sync.dma_start(out=ap_[:], in_=apD[:, :])

                msgs = sp.tile([P, sc * 4 * B], F32, tag="msgs")
                m3 = msgs[:].rearrange("p (c e) -> p c e", e=4 * B)
                nc.gpsimd.dma_gather(
                    m3, xqD[:, :], gt[:], nidx, nidx, 4 * B,
                    single_packet=False, queue_num=qorder[si % 4])

                m2 = msgs[:].rearrange("p (q b) -> p q b", b=B)
                w4b = wt[:].unsqueeze(-1).to_broadcast([P, sc * 4, B])
                nc.vector.tensor_mul(m2, m2, w4b)

                # fused (sub-row x degree-window) reduce: a group's window
                # block is contiguous with uniform stride B over (d, s).
                for (ga, gb, d) in sl["runs"]:
                    src_ap = (msgs[:, (int(S[ga]) - A) * 4 * B:
                              (int(S[gb]) - A) * 4 * B]
                              .rearrange("p (n dd b) -> p n b dd",
                                         dd=4 * d, b=B))
                    dst_ap = agg[:, ga * B:gb * B].rearrange(
                        "p (n b) -> p n b", b=B)
                    nc.vector.tensor_reduce(
                        dst_ap, src_ap, axis=mybir.AxisListType.X,
                        op=mybir.AluOpType.add)
                for (g, a, b, k) in sl["partials"]:
                    src_ap = (msgs[:, (a - A) * 4 * B:(b - A) * 4 * B]
                              .rearrange("p (n dd b) -> p n b dd",
                                         dd=4 * (b - a), b=B))
                    dst = shadows[k][:, g * B:(g + 1) * B]
                    nc.vector.tensor_reduce(
                        dst.rearrange("p (n b) -> p n b", b=B), src_ap,
                        axis=mybir.AxisListType.X, op=mybir.AluOpType.add)

                for (mk, g_lo, g_hi) in echunks or []:
                    if mk != si:
                        continue
                    # out[:, glo:ghi] = clip(A + C*tanh(agg+sh+bias), 0, cap)
                    ng = g_hi - g_lo
                    av = agg[:, g_lo * B:g_hi * B]
                    a3 = av.rearrange("p (g b) -> p g b", b=B)
                    for sh in shadows:
                        nc.vector.tensor_add(av, av,
                                             sh[:, g_lo * B:g_hi * B])
                    bib = (bi[:, g_lo:g_hi].unsqueeze(-1)
                           .to_broadcast([P, ng, B]))
                    cmb = (cm[:, g_lo:g_hi].unsqueeze(-1)
                           .to_broadcast([P, ng, B]))
                    cpb = (cp[:, g_lo:g_hi].unsqueeze(-1)
                           .to_broadcast([P, ng, B]))
                    nc.vector.tensor_add(a3, a3, bib)
                    nc.scalar.activation(av, av, Tanh)
                    nc.vector.tensor_mul(a3, a3, cmb)
                    nc.vector.tensor_add(av, av, ap_[:, g_lo * B:g_hi * B])
                    nc.vector.tensor_scalar_max(av, av, 0.0)
                    nc.vector.tensor_tensor(out=a3, in0=a3, in1=cpb,
                                            op=mybir.AluOpType.min)
                    nc.sync.dma_start(out=outD[:, g_lo * B:g_hi * B], in_=av)

            pass

    nc.compile()
    return nc


# ---------------------------------------------------------------------------
# Entry point
# ---------------------------------------------------------------------------

def kernel(x, theta_graph, node_bias, rate_log_scale, base_rate, capacity,
           sign_prior, conf_scale, delay_scale, src_index, dst_index,
           learn_mask):
    ins, plan = _prep(x, theta_graph, node_bias, rate_log_scale, base_rate,
                      capacity, sign_prior, conf_scale, delay_scale,
                      src_index, dst_index, learn_mask, NCORES)
    nc = build(plan["B"], plan["G"], plan["F"], plan["D"], plan["S"],
               plan["slabs"], plan["n4"], plan["nshadow"], plan["echunks"])
    res = run_bass_kernel_spmd(nc, ins, core_ids=list(range(NCORES)))
    return _assemble(res.results, plan)



# revision 15
# speedup vs baseline: 1.5500x; 1.5500x over previous
"""Trainium2 Bass kernel for DifferentiableSupergraphDynamics.

Computation:
    edge_w = where(learn_mask, tanh(theta), sign*conf) * delay      [E]
    msgs   = x[:, src] * edge_w                                     [B, E]
    agg    = scatter_add(msgs -> dst)                               [B, N]
    rate   = base_rate * exp(rate_log_scale)                        [N]
    drive  = tanh(agg + bias)
    x_next = clip(x + DT * rate * (drive*cap - x), 0, cap)

Design (v2):
  - Destination nodes are dealt round-robin by total-degree rank across the
    8 cores; every edge lives on its destination's core (no collective).
  - Single-structure gather: the x table is packed as [N/4, 4*B] f32 (4 node
    rows per 256B line).  Each edge's SWDGE descriptor fetches the full 256B
    line holding its source row, so the int16 gather index (= src//4 <
    32768) reaches the whole table and no src-range structure split / merge
    scatter is needed.  Row selection happens in the weighted reduce: a
    host-built [4]-wide weight mask per slot (w at position src%4, else 0).
  - CSR: per-core nodes sorted by degree, groups of 128 partitions padded to
    the group max degree D (D shared across cores).  The slot space is cut
    into uniform STRIP_COLS-column slabs (2048-slot gather calls, SWDGE
    queues round-robin 1,2,3,0 -- equal sizes keep queue 0's
    pipeline-blocking ucode hidden under queues 1-3's generation).  Per
    slab, one DVE mask-multiply and one fused (sub-row x degree-window)
    reduce run under later slabs' gathers; group windows straddling a slab
    boundary reduce each piece directly into a per-piece-index shadow
    aggregate (no read-modify-write), merged once at the end.
  - Edge weights and the epilogue's affine terms are precomputed on host:
    out = clip(A + C*tanh(agg + bias), 0, cap) with A=(1-DT*rate)*x and
    C = DT*rate*cap.  The epilogue runs in two chunks (groups finalized by
    mid-kernel, rest at the end).
"""

import numpy as np

import concourse.bass as bass
import concourse.bacc as bacc
import concourse.mybir as mybir
import concourse.tile as tile
from concourse.bass_utils import run_bass_kernel_spmd

P = 128
NCORES = 8
DT = 0.1
EPS = 1e-5
STRIP_COLS = 16            # slab width in columns (4096-slot gather calls)
MAX_CALL = 8192            # SWDGE ring capacity per call

F32 = mybir.dt.float32
I16 = mybir.dt.int16


def _wrap_idx(flat):
    """SWDGE wrapped int16 index layout for one call: index j at
    [j%16, j//16], replicated to 128 partitions."""
    n = len(flat)
    assert n % 16 == 0
    cols = flat.reshape(n // 16, 16).T
    return np.concatenate([cols] * 8, axis=0)


# ---------------------------------------------------------------------------
# Host-side data preparation
# ---------------------------------------------------------------------------

def _prep(x, theta, bias, ratelog, baserate, cap, sign, conf, delay, src, dst,
          mask, n_cores):
    B, N = x.shape
    E = src.shape[0]

    src = np.asarray(src).astype(np.int64)
    dst = np.asarray(dst).astype(np.int64)
    x = np.asarray(x, dtype=np.float32)

    # host-computed edge weight
    w = np.where(np.asarray(mask).astype(bool),
                 np.tanh(np.asarray(theta, dtype=np.float32)),
                 np.asarray(sign, dtype=np.float32) *
                 np.asarray(conf, dtype=np.float32)) \
        * np.asarray(delay, dtype=np.float32)

    deg = np.bincount(dst, minlength=N)
    order = np.argsort(-deg, kind="stable")
    rank_of = np.empty(N, dtype=np.int64)
    rank_of[order] = np.arange(N)
    core_of = rank_of % n_cores
    pos_of = rank_of // n_cores
    npc = (N + n_cores - 1) // n_cores
    G = (npc + P - 1) // P
    nper = G * P

    # shared-over-cores group degree D[g] = max over cores of group max
    degs = np.zeros((n_cores, nper), dtype=np.int64)
    for c in range(n_cores):
        dc = deg[order[c::n_cores]]
        degs[c, :len(dc)] = dc
    D = degs.reshape(n_cores, G, P).max(axis=(0, 2))
    S = np.zeros(G + 1, dtype=np.int64)
    S[1:] = np.cumsum(D)
    F = int(S[-1])

    # uniform slabs of STRIP_COLS columns (cut anywhere; group windows that
    # straddle a slab boundary are reduced partially and accumulated)
    slabs = []
    piece_no = {}
    A = 0
    while A < F:
        wcols = STRIP_COLS if A < F - 128 else STRIP_COLS // 2
        Bc = min(A + wcols, F)
        runs = []          # (ga, gb, d): full windows, equal-d batches
        partials = []      # (g, a, b): partial window cols [a, b)
        fulls = []
        for g in range(G):
            if D[g] == 0 or S[g] >= Bc or S[g + 1] <= A:
                continue
            a, b = max(int(S[g]), A), min(int(S[g + 1]), Bc)
            if a == S[g] and b == S[g + 1]:
                fulls.append(g)
            else:
                k = piece_no.get(g, 0)
                piece_no[g] = k + 1
                partials.append((g, a, b, k))
        i = 0
        while i < len(fulls):
            j = i + 1
            while (j < len(fulls) and fulls[j] == fulls[j - 1] + 1
                   and D[fulls[j]] == D[fulls[i]]):
                j += 1
            runs.append((fulls[i], fulls[j - 1] + 1, int(D[fulls[i]])))
            i = j
        slabs.append(dict(A=A, B=Bc, runs=runs, partials=partials))
        A = Bc

    # edge -> slot
    ec = core_of[dst]
    ep = pos_of[dst]
    eord = np.argsort(ec * nper + ep, kind="stable")
    key = (ec * nper + ep)[eord]
    key_change = np.ones(E, dtype=bool)
    key_change[1:] = key[1:] != key[:-1]
    run_starts = np.flatnonzero(key_change)
    occ = np.arange(E) - run_starts[np.cumsum(key_change) - 1]
    g = ep[eord] // P
    pp = ep[eord] % P
    col = S[g] + occ
    slot_i = pp + P * col

    n4 = (N + 3) // 4
    srcg = (src[eord] // 4).astype(np.int16)
    subr = (src[eord] % 4).astype(np.int64)
    assert n4 <= 32768

    idxA = np.zeros((n_cores, F * P), np.int16)
    w4A = np.zeros((n_cores, P, F, 4), np.float32)
    ecs = ec[eord]
    idxA[ecs, slot_i] = srcg
    w4A[ecs, pp, col, subr] = w[eord]

    # wrapped gather indices (whole array; slab slices are column slices)
    gidx = np.zeros((n_cores, 128, F * 8), np.int16)
    for c in range(n_cores):
        gidx[c] = _wrap_idx(idxA[c])

    # node params in canonical [P, G] placement
    rate = np.asarray(baserate, dtype=np.float32) * \
        np.exp(np.asarray(ratelog, dtype=np.float32))
    Cv = DT * rate * np.asarray(cap, dtype=np.float32)
    Av = (1.0 - DT * rate)[None, :] * x            # [B, N]

    biasA = np.zeros((n_cores, P, G), np.float32)
    CA = np.zeros((n_cores, P, G), np.float32)
    capA = np.ones((n_cores, P, G), np.float32)
    AA = np.zeros((n_cores, P, G, B), np.float32)
    node_ids = np.full((n_cores, P, G), -1, np.int64)
    biasv = np.asarray(bias, dtype=np.float32)
    capv = np.asarray(cap, dtype=np.float32)
    for c in range(n_cores):
        nd = order[c::n_cores]                     # nodes at pos 0..len-1
        j = np.arange(len(nd))
        pidx = (j % P, j // P)
        node_ids[c][pidx] = nd
        biasA[c][pidx] = biasv[nd]
        CA[c][pidx] = Cv[nd]
        capA[c][pidx] = capv[nd]
        AA[c][pidx[0], pidx[1], :] = Av[:, nd].T

    xq = np.zeros((n4, 4 * B), np.float32)
    xq.reshape(-1, B)[:N] = x.T

    ins = []
    for c in range(n_cores):
        ins.append({
            "xq": xq,
            "gidx": gidx[c],
            "w4": w4A[c].reshape(P, F * 4),
            "bias": biasA[c],
            "cmul": CA[c],
            "cap": capA[c],
            "apre": AA[c].reshape(P, G * B),
        })
    nshadow = max(piece_no.values(), default=1)
    # epilogue chunks: after slab `si`, groups [g_lo, g_hi) are final
    sends = np.searchsorted([sl["B"] for sl in slabs],
                            np.maximum(S[1:] - 1, S[:-1]), side="right")
    sends[D == 0] = len(slabs) - 1
    nch = 3
    marks = sorted(set(round((i + 1) * len(slabs) / nch) - 1
                       for i in range(nch)))
    echunks = []
    g_done = 0
    for mk in marks:
        hi = int(np.searchsorted(sends, mk, side="right"))
        if mk == len(slabs) - 1:
            hi = G
        if hi > g_done:
            echunks.append((int(mk), g_done, hi))
            g_done = hi
    plan = dict(B=B, G=G, F=F, D=D, S=S, slabs=slabs, n4=n4,
                nshadow=nshadow, echunks=echunks, node_ids=node_ids)
    return ins, plan


def _assemble(results, plan):
    B, G = plan["B"], plan["G"]
    N = 0
    for nid in plan["node_ids"]:
        N = max(N, nid.max() + 1)
    out = np.empty((B, N), dtype=np.float32)
    for ci, res in enumerate(results):
        o = res["out"].reshape(P, G, B)
        nid = plan["node_ids"][ci]
        ok = nid >= 0
        out[:, nid[ok]] = o[ok].T
    return out


# ---------------------------------------------------------------------------
# Device kernel
# ---------------------------------------------------------------------------

def _equal_d_runs(D, g0, g1):
    runs = []
    a = g0
    while a < g1:
        b = a + 1
        while b < g1 and D[b] == D[a]:
            b += 1
        runs.append((a, b, int(D[a])))
        a = b
    return runs


def build(B, G, F, D, S, slabs, n4, nshadow=2, echunks=None):
    nc = bacc.Bacc("TRN2", target_bir_lowering=False, debug=False,
                   enable_asserts=False, num_swdge_queues=4)

    xqD = nc.dram_tensor("xq", [n4, 4 * B], F32, kind="ExternalInput")
    giD = nc.dram_tensor("gidx", [128, F * 8], I16, kind="ExternalInput")
    w4D = nc.dram_tensor("w4", [P, F * 4], F32, kind="ExternalInput")
    biD = nc.dram_tensor("bias", [P, G], F32, kind="ExternalInput")
    cmD = nc.dram_tensor("cmul", [P, G], F32, kind="ExternalInput")
    cpD = nc.dram_tensor("cap", [P, G], F32, kind="ExternalInput")
    apD = nc.dram_tensor("apre", [P, G * B], F32, kind="ExternalInput")
    outD = nc.dram_tensor("out", [P, G * B], F32, kind="ExternalOutput")

    Tanh = mybir.ActivationFunctionType.Tanh
    qorder = [1, 2, 3, 0]

    with tile.TileContext(nc) as tc:
        with (
            tc.tile_pool(name="persist", bufs=1) as ppool,
            tc.tile_pool(name="strip", bufs=12) as sp,
        ):
            agg = ppool.tile([P, G * B], F32, tag="agg")
            nc.vector.memset(agg[:], 0.0)
            shadows = []
            for k in range(nshadow):
                sh = ppool.tile([P, G * B], F32, tag=f"shadow{k}")
                nc.vector.memset(sh[:], 0.0)
                shadows.append(sh)

            # epilogue params up front so the tail never waits on DMA
            bi = ppool.tile([P, G], F32, tag="bi")
            cm = ppool.tile([P, G], F32, tag="cm")
            cp = ppool.tile([P, G], F32, tag="cp")
            ap_ = ppool.tile([P, G * B], F32, tag="ap")

            for si, sl in enumerate(slabs):
                A, Bc = sl["A"], sl["B"]
                sc = Bc - A
                nidx = sc * P
                gt = sp.tile([128, sc * 8], I16, tag="gidx")
                nc.sync.dma_start(out=gt[:], in_=giD[:, A * 8:Bc * 8])
                wt = sp.tile([P, sc * 4], F32, tag="w4")
                nc.sync.dma_start(out=wt[:], in_=w4D[:, A * 4:Bc * 4])
                if si == 0:
                    nc.sync.dma_start(out=bi[:], in_=biD[:, :])
                    nc.sync.dma_start(out=cm[:], in_=cmD[:, :])
                    nc.sync.dma_start(out=cp[:], in_=cpD[:, :])
                    nc.sync.dma_start(out=ap_[:], in_=apD[:, :])

                msgs = sp.tile([P, sc * 4 * B], F32, tag="msgs")
                m3 = msgs[:].rearrange("p (c e) -> p c e", e=4 * B)
                nc.gpsimd.dma_gather(
                    m3, xqD[:, :], gt[:], nidx, nidx, 4 * B,
                    single_packet=False, queue_num=qorder[si % 4])

                m2 = msgs[:].rearrange("p (q b) -> p q b", b=B)
                w4b = wt[:].unsqueeze(-1).to_broadcast([P, sc * 4, B])
                nc.vector.tensor_mul(m2, m2, w4b)

                # fused (sub-row x degree-window) reduce: a group's window
                # block is contiguous with uniform stride B over (d, s).
                for (ga, gb, d) in sl["runs"]:
                    src_ap = (msgs[:, (int(S[ga]) - A) * 4 * B:
                              (int(S[gb]) - A) * 4 * B]
                              .rearrange("p (n dd b) -> p n b dd",
                                         dd=4 * d, b=B))
                    dst_ap = agg[:, ga * B:gb * B].rearrange(
                        "p (n b) -> p n b", b=B)
                    nc.vector.tensor_reduce(
                        dst_ap, src_ap, axis=mybir.AxisListType.X,
                        op=mybir.AluOpType.add)
                for (g, a, b, k) in sl["partials"]:
                    src_ap = (msgs[:, (a - A) * 4 * B:(b - A) * 4 * B]
                              .rearrange("p (n dd b) -> p n b dd",
                                         dd=4 * (b - a), b=B))
                    dst = shadows[k][:, g * B:(g + 1) * B]
                    nc.vector.tensor_reduce(
                        dst.rearrange("p (n b) -> p n b", b=B), src_ap,
                        axis=mybir.AxisListType.X, op=mybir.AluOpType.add)

                for (mk, g_lo, g_hi) in echunks or []:
                    if mk != si:
                        continue
                    # out[:, glo:ghi] = clip(A + C*tanh(agg+sh+bias), 0, cap)
                    ng = g_hi - g_lo
                    av = agg[:, g_lo * B:g_hi * B]
                    a3 = av.rearrange("p (g b) -> p g b", b=B)
                    for sh in shadows:
                        nc.vector.tensor_add(av, av,
                                             sh[:, g_lo * B:g_hi * B])
                    bib = (bi[:, g_lo:g_hi].unsqueeze(-1)
                           .to_broadcast([P, ng, B]))
                    cmb = (cm[:, g_lo:g_hi].unsqueeze(-1)
                           .to_broadcast([P, ng, B]))
                    cpb = (cp[:, g_lo:g_hi].unsqueeze(-1)
                           .to_broadcast([P, ng, B]))
                    nc.vector.tensor_add(a3, a3, bib)
                    nc.scalar.activation(av, av, Tanh)
                    nc.vector.tensor_mul(a3, a3, cmb)
                    nc.vector.tensor_add(av, av, ap_[:, g_lo * B:g_hi * B])
                    nc.vector.tensor_scalar_max(av, av, 0.0)
                    nc.vector.tensor_tensor(out=a3, in0=a3, in1=cpb,
                                            op=mybir.AluOpType.min)
                    nc.sync.dma_start(out=outD[:, g_lo * B:g_hi * B], in_=av)

            pass

    nc.compile()
    return nc


# ---------------------------------------------------------------------------
# Entry point
# ---------------------------------------------------------------------------

def kernel(x, theta_graph, node_bias, rate_log_scale, base_rate, capacity,
           sign_prior, conf_scale, delay_scale, src_index, dst_index,
           learn_mask):
    ins, plan = _prep(x, theta_graph, node_bias, rate_log_scale, base_rate,
                      capacity, sign_prior, conf_scale, delay_scale,
                      src_index, dst_index, learn_mask, NCORES)
    nc = build(plan["B"], plan["G"], plan["F"], plan["D"], plan["S"],
               plan["slabs"], plan["n4"], plan["nshadow"], plan["echunks"])
    res = run_bass_kernel_spmd(nc, ins, core_ids=list(range(NCORES)))
    return _assemble(res.results, plan)

